# revision 24
# baseline (speedup 1.0000x reference)
"""Causal self-attention (L=8192, D=2048) on 8 TRN2 NeuronCores.

Sharding: core c owns eight 128-row query blocks with global ids {8p+c,
p=0..7} and KV rows [c*1024, (c+1)*1024).  For each 128-key tile the set of
positions still inside their causal range is a contiguous suffix of the
qt_sb columns, so one wide score/PV matmul (N up to 512) covers all of them
at once: causal over-compute drops to 1.125x of the ideal triangle with no
small-N matmul penalty.  Everything flows in bf16 (fp32 PSUM accumulation): the host passes
x^T/z^T pre-transposed bf16 so the device does no transposes, and K^T/V are
gathered in bf16, halving DMA and collective bytes.  The K^T and V AllGathers
are each split in two key-halves (via jb-outer / row-group-outer projection
loops) so four small collectives pipeline behind the remaining projection
compute instead of serializing at the phase boundary.  Phase 2
iterates chunk-outer/block-inner so each K/V chunk is loaded once per core;
scores are computed transposed (S^T = K^T-tile @ Q^T) so the softmax sum
reduces via a ones-matmul on the PE and P^T feeds the P@V matmul directly.
Each position finalizes (1/l scale + bv + store) right after its last causal
chunk, overlapping the epilogue with later chunks' compute.  No
max-subtraction: scores/sqrt(d) are O(+-6) for these inputs, well within exp
range.
"""

import math
import time
from contextlib import ExitStack

import ml_dtypes
import numpy as np

import concourse.bass as bass
import concourse.tile as tile
from concourse import bacc, mybir
from concourse.bass_utils import run_bass_kernel_spmd

L = 8192
D = 2048  # d_x == d_attn == d_v
NCORES = 8
QB = 128  # query rows per block
NBLK = 8  # q-blocks per core
JC = 256  # kv keys per chunk
NCH = L // JC  # 32 chunks
NDT = D // 128  # 16 contraction tiles
SCALE = 1.0 / math.sqrt(D)

F32 = mybir.dt.float32
BF16 = mybir.dt.bfloat16

_cache = {}
_SIM_AG_TRAFFIC = True


def _build(repeat=1, sim=False):
    nc = bacc.Bacc("TRN2", num_devices=1 if sim else NCORES)

    xT = nc.dram_tensor("xT_blk", [D, NBLK * QB], BF16, kind="ExternalInput")
    zT = nc.dram_tensor("zT_blk", [D, 1024], BF16, kind="ExternalInput")
    wq = nc.dram_tensor("wq", [D, D], BF16, kind="ExternalInput")
    wk = nc.dram_tensor("wk", [D, D], BF16, kind="ExternalInput")
    wv = nc.dram_tensor("wv", [D, D], BF16, kind="ExternalInput")
    bq = nc.dram_tensor("bq", [D], F32, kind="ExternalInput")
    bk = nc.dram_tensor("bk", [D], F32, kind="ExternalInput")
    bv = nc.dram_tensor("bv", [D], F32, kind="ExternalInput")
    ig_rows = nc.dram_tensor("ig_rows", [NBLK, QB], F32, kind="ExternalInput")
    out = nc.dram_tensor("out", [NBLK * QB, D], F32, kind="ExternalOutput")

    shared = {} if sim else dict(addr_space="Shared")
    # K^T stored chunk-major: [chunk][d-sub 128][d-tile][key].  Split in two
    # key-halves per core so each half AllGathers as soon as it is projected.
    kt_locs = [
        nc.dram_tensor(f"kt_loc{h}", [2, 128, NDT, JC], BF16) for h in range(2)
    ]
    kt_gs = [
        nc.dram_tensor(f"kt_g{h}", [2 * NCORES, 128, NDT, JC], BF16, **shared)
        for h in range(2)
    ]
    # V split in key-row halves (local z rows 0:512 / 512:1024) so the first
    # half's gather overlaps the second half's projection, and attention's
    # early chunk pairs need only half A
    v_locs = [nc.dram_tensor(f"v_loc{h}", [512, D], BF16) for h in range(2)]
    v_gs = [
        nc.dram_tensor(f"v_g{h}", [L // 2, D], BF16, **shared) for h in range(2)
    ]

    def allgather(src, dst):
        if sim:
            if _SIM_AG_TRAFFIC:
                # emulate the gather's HBM traffic + dependency structure
                # (real collectives run on reserved SDMA rows)
                n = src.shape[0]
                for r in range(NCORES):
                    nc.gpsimd.dma_start(dst[r * n : (r + 1) * n], src[:])
            return
        nc.gpsimd.collective_compute(
            "AllGather",
            mybir.AluOpType.bypass,
            replica_groups=[list(range(NCORES))],
            ins=[src.ap().opt()],
            outs=[dst.ap().opt()],
        )

    with tile.TileContext(nc) as tc:
        with ExitStack() as consts:
            cp = consts.enter_context(tc.tile_pool(name="consts", bufs=1))
            ones_f = cp.tile([128, 2], F32)
            nc.vector.memset(ones_f, 1.0)
            ones = cp.tile([128, 2], BF16)
            nc.vector.tensor_copy(ones, ones_f)
            # jg[p, k] = 128*k + p == global j index of partition p in j-tile k
            jg = cp.tile([128, L // 128], F32)
            nc.gpsimd.iota(
                jg,
                pattern=[[128, L // 128]],
                base=0,
                channel_multiplier=1,
                allow_small_or_imprecise_dtypes=True,
            )
            # ig[p][:, f] = global i index of column f of block p (same all parts)
            igs = []
            for p in range(NBLK):
                igt = cp.tile([128, QB], F32, tag=f"ig{p}")
                nc.gpsimd.dma_start(
                    igt,
                    bass.AP(tensor=ig_rows, offset=p * QB, ap=[[0, 128], [1, QB]]),
                )
                igs.append(igt)
            bv_bc = cp.tile([128, D], F32)
            nc.gpsimd.dma_start(
                bv_bc, bass.AP(tensor=bv, offset=0, ap=[[0, 128], [1, D]])
            )
            bq_sb = cp.tile([128, NDT], F32, tag="bq")
            nc.gpsimd.dma_start(
                bq_sb, bass.AP(tensor=bq, offset=0, ap=[[1, 128], [128, NDT]])
            )
            bk_sb = cp.tile([128, NDT], F32, tag="bk")
            nc.gpsimd.dma_start(
                bk_sb, bass.AP(tensor=bk, offset=0, ap=[[1, 128], [128, NDT]])
            )
            qtp = consts.enter_context(tc.tile_pool(name="qt_sb", bufs=1))

            for _rep in range(repeat):
                qt_sb = qtp.tile([128, NDT, NBLK * QB], BF16, tag="qt")

                # ---------------- Phase 1: projections ----------------
                with ExitStack() as p1:
                    ztp = p1.enter_context(tc.tile_pool(name="zt", bufs=1))
                    xtp = p1.enter_context(tc.tile_pool(name="xt", bufs=1))
                    prp = p1.enter_context(
                        tc.tile_pool(name="prj_ps", bufs=3, space="PSUM")
                    )
                    stg = p1.enter_context(tc.tile_pool(name="stg", bufs=4))
                    wpp = p1.enter_context(tc.tile_pool(name="wpanel", bufs=2))

                    # first weight panel before the bulk loads: PE starts ~2us in
                    wp0 = wpp.tile([128, NDT, 256], BF16, tag="wp")
                    nc.sync.dma_start(
                        wp0, wk[:, 0:256].rearrange("(dt p) c -> p dt c", p=128)
                    )
                    # z^T / x^T resident, loaded in d-tile quarters so the first
                    # projection group starts as soon as its slices land
                    zt = ztp.tile([128, NDT, 1024], BF16)
                    for jh in range(2):
                        # half 0 feeds the first jb=0 groups (sync ring, right
                        # behind wp0); half 1 rides the idle SWDGE ring so the
                        # jb=0 weight panels stream uninterrupted
                        eng = nc.sync if jh == 0 else nc.gpsimd
                        eng.dma_start(
                            zt[:, :, 512 * jh : 512 * (jh + 1)],
                            zT[:, 512 * jh : 512 * (jh + 1)].rearrange(
                                "(t p) j -> p t j", p=128
                            ),
                        )
                    def proj_T(w_dram, b_sb, rhs3, jb, kt_dst=None, qt_dst=None,
                               first_wp=None):
                        # dst[t*128:(t+1)*128, :] = (W[:, t-cols]^T @ rhs_jb) + b[t]
                        for t2 in range(NDT // 2):
                            if first_wp is not None and t2 == 0:
                                wp = first_wp
                            else:
                                wp = wpp.tile([128, NDT, 256], BF16, tag="wp")
                                nc.sync.dma_start(
                                    wp,
                                    w_dram[:, t2 * 256 : (t2 + 1) * 256].rearrange(
                                        "(dt p) c -> p dt c", p=128
                                    ),
                                )
                            for th in range(2):
                                t = 2 * t2 + th
                                ps = prp.tile([128, 512], F32, tag="prj")
                                for dt in range(NDT):
                                    nc.tensor.matmul(
                                        ps,
                                        wp[:, dt, th * 128 : (th + 1) * 128],
                                        rhs3[:, dt, jb * 512 : (jb + 1) * 512],
                                        start=(dt == 0),
                                        stop=(dt == NDT - 1),
                                    )
                                if kt_dst is not None:
                                    st = stg.tile([128, 512], BF16, tag="stg")
                                    nc.scalar.activation(
                                        st,
                                        ps,
                                        mybir.ActivationFunctionType.Identity,
                                        bias=b_sb[:, t : t + 1],
                                    )
                                    for h in range(2):
                                        nc.scalar.dma_start(
                                            kt_dst[h][:, t, :],
                                            st[:, h * JC : (h + 1) * JC],
                                        )
                                else:
                                    nc.scalar.activation(
                                        qt_dst[:, t, jb * 512 : (jb + 1) * 512],
                                        ps,
                                        mybir.ActivationFunctionType.Identity,
                                        bias=b_sb[:, t : t + 1],
                                    )

                    # K first, in key-halves, so each half's gather overlaps
                    # the remaining projections
                    proj_T(wk, bk_sb, zt, jb=0, kt_dst=kt_locs[0], first_wp=wp0)
                    allgather(kt_locs[0], kt_gs[0])
                    proj_T(wk, bk_sb, zt, jb=1, kt_dst=kt_locs[1])
                    allgather(kt_locs[1], kt_gs[1])

                    # V = z @ Wv (natural layout); bv folded into the
                    # epilogue.  Row-group outer (panels reloaded per group)
                    # so each 512-key half gathers as soon as it is done.
                    for jg2 in range(2):
                        for cp4 in range(4):
                            wvh = wpp.tile([128, NDT, 512], BF16, tag="wvh")
                            nc.sync.dma_start(
                                wvh,
                                wv[:, cp4 * 512 : (cp4 + 1) * 512].rearrange(
                                    "(dt p) c -> p dt c", p=128
                                ),
                            )
                            for jt in range(4):
                                jtg = jg2 * 4 + jt
                                ps = prp.tile([128, 512], F32, tag="prj")
                                for dt in range(NDT):
                                    nc.tensor.matmul(
                                        ps,
                                        zt[:, dt, jtg * 128 : (jtg + 1) * 128],
                                        wvh[:, dt, :],
                                        start=(dt == 0),
                                        stop=(dt == NDT - 1),
                                    )
                                st = stg.tile([128, 512], BF16, tag="stg")
                                nc.vector.tensor_copy(st, ps)
                                nc.scalar.dma_start(
                                    v_locs[jg2][
                                        jt * 128 : (jt + 1) * 128,
                                        cp4 * 512 : (cp4 + 1) * 512,
                                    ],
                                    st,
                                )
                        allgather(v_locs[jg2], v_gs[jg2])

                    # Q last: its output stays in SBUF for phase 2.  x^T is
                    # loaded here (not earlier) so its 4MB doesn't clog the
                    # DMA queue ahead of the K/V weight panels.
                    xt = xtp.tile([128, NDT, NBLK * QB], BF16)
                    for q in range(4):
                        nc.sync.dma_start(
                            xt[:, 4 * q : 4 * (q + 1), :],
                            xT[512 * q : 512 * (q + 1), :].rearrange(
                                "(t p) j -> p t j", p=128
                            ),
                        )
                    for jb in range(2):
                        proj_T(wq, bq_sb, xt, jb=jb, qt_dst=qt_sb)

                # ---------------- Phase 2: causal attention ----------------
                with ExitStack() as p2:
                    ktp = p2.enter_context(tc.tile_pool(name="kt", bufs=4))
                    vcp = p2.enter_context(tc.tile_pool(name="vc", bufs=4))
                    ptp = p2.enter_context(tc.tile_pool(name="pt", bufs=10))
                    mkp = p2.enter_context(tc.tile_pool(name="mk", bufs=2))
                    acp = p2.enter_context(tc.tile_pool(name="acc", bufs=1))
                    fin = p2.enter_context(tc.tile_pool(name="fin", bufs=2))
                    stp = p2.enter_context(
                        tc.tile_pool(name="st_ps", bufs=4, space="PSUM")
                    )
                    pvp = p2.enter_context(
                        tc.tile_pool(name="pv_ps", bufs=3, space="PSUM")
                    )
                    llp = p2.enter_context(
                        tc.tile_pool(name="l_ps", bufs=1, space="PSUM")
                    )

                    accs = [
                        acp.tile([128, D], F32, tag=f"acc{r}", name=f"acc{r}")
                        for r in range(NBLK)
                    ]
                    l_acc = acp.tile([128, 2 * NBLK], F32, tag="lacc")

                    recips = {}

                    def finalize_dh(p, dh):
                        # ACT/DVE pipeline in 512-col pieces; a contiguous
                        # half-row store on the sync ring (idle at tail),
                        # emitted right after the PV spill completing it
                        if p not in recips:
                            recip = fin.tile([128, 2], F32, tag="recip")
                            nc.vector.reciprocal(
                                recip, l_acc[:, 2 * p : 2 * p + 2]
                            )
                            recips[p] = recip
                        recip = recips[p]
                        of = fin.tile([128, D // 2], F32, tag="of")
                        for dq in range(2):
                            sl = slice(
                                dh * 1024 + dq * 512,
                                dh * 1024 + (dq + 1) * 512,
                            )
                            osl = slice(dq * 512, (dq + 1) * 512)
                            nc.scalar.activation(
                                of[:, osl],
                                accs[p][:, sl],
                                mybir.ActivationFunctionType.Copy,
                                scale=recip[:, 0:1],
                            )
                            nc.vector.tensor_add(
                                of[:, osl], of[:, osl], bv_bc[:, sl]
                            )
                        nc.sync.dma_start(
                            out[
                                p * QB : (p + 1) * QB,
                                dh * 1024 : (dh + 1) * 1024,
                            ],
                            of,
                        )

                    for m in range(NCH // 2):
                        # group m: stored chunks (2m, 2m+1) = j-tiles
                        # [4m, 4m+4), all with the same active-position set
                        # [pmin, 8) since the range never crosses an
                        # 8-j-tile boundary
                        pmin = m // 2
                        W = (NBLK - pmin) * QB
                        nW = (W + 511) // 512
                        js = (2 * m, 2 * m + 1)
                        kts = []
                        vcs = []
                        for ci, j in enumerate(js):
                            kt = ktp.tile([128, NDT, JC], BF16, tag="kt")
                            nc.sync.dma_start(
                                kt, kt_gs[(j % 4) // 2][2 * (j // 4) + (j % 2)]
                            )
                            kts.append(kt)
                            mj = j % 4
                            ro = 512 * (j // 4) + 256 * (mj % 2)
                            vch = vcp.tile([128, 2, D], BF16, tag="vc", name="vc")
                            nc.sync.dma_start(
                                vch,
                                v_gs[mj // 2][ro : ro + 256, :].rearrange(
                                    "(jt p) d -> p jt d", p=128
                                ),
                            )
                            vcs.append(vch)
                        # scores + exp + mask for all 4 j-tiles, fused over
                        # the active positions (one wide MM per 512 piece)
                        pts = []  # pts[2*ci+jt][nh]
                        for ci in range(2):
                            for jt in range(2):
                                pieces = []
                                for nh in range(nW):
                                    w = min(512, W - 512 * nh)
                                    st_ps = stp.tile([128, 512], F32, tag="st")
                                    q0 = pmin * QB + 512 * nh
                                    for dt in range(NDT):
                                        nc.tensor.matmul(
                                            st_ps[:, :w],
                                            kts[ci][
                                                :, dt, jt * 128 : (jt + 1) * 128
                                            ],
                                            qt_sb[:, dt, q0 : q0 + w],
                                            start=(dt == 0),
                                            stop=(dt == NDT - 1),
                                        )
                                    pt = ptp.tile([128, 512], BF16, tag="pt")
                                    nc.scalar.activation(
                                        pt[:, :w],
                                        st_ps[:, :w],
                                        mybir.ActivationFunctionType.Exp,
                                        scale=SCALE,
                                    )
                                    if nh == 0:
                                        # position pmin is inside its causal
                                        # band for every j-tile of this group
                                        k = 4 * m + 2 * ci + jt
                                        mk = mkp.tile([128, QB], BF16, tag="mk")
                                        nc.vector.tensor_scalar(
                                            mk,
                                            igs[pmin],
                                            jg[:, k : k + 1],
                                            None,
                                            mybir.AluOpType.is_ge,
                                        )
                                        nc.vector.tensor_mul(
                                            pt[:, :QB], pt[:, :QB], mk
                                        )
                                    pieces.append(pt)
                                pts.append(pieces)

                        def ptsl(idx, r):
                            off = (r - pmin) * QB
                            return pts[idx][off // 512][
                                :, off % 512 : off % 512 + QB
                            ]

                        for r in range(pmin, NBLK):
                            l_ps = llp.tile([128, 2], F32, tag="l")
                            for idx in range(4):
                                nc.tensor.matmul(
                                    l_ps,
                                    ptsl(idx, r),
                                    ones,
                                    start=(idx == 0),
                                    stop=(idx == 3),
                                )
                            dst = l_acc[:, 2 * r : 2 * r + 2]
                            if m == 0:
                                nc.vector.tensor_copy(dst, l_ps)
                            else:
                                nc.vector.tensor_add(dst, dst, l_ps)
                        for r in range(pmin, NBLK):
                            for dvc in range(4):
                                pv = pvp.tile([128, 512], F32, tag="pv")
                                for ci in range(2):
                                    for jt in range(2):
                                        nc.tensor.matmul(
                                            pv,
                                            ptsl(2 * ci + jt, r),
                                            vcs[ci][
                                                :,
                                                jt,
                                                dvc * 512 : (dvc + 1) * 512,
                                            ],
                                            start=(ci == 0 and jt == 0),
                                            stop=(ci == 1 and jt == 1),
                                        )
                                dst = accs[r][:, dvc * 512 : (dvc + 1) * 512]
                                if m == 0:
                                    nc.vector.tensor_copy(dst, pv)
                                else:
                                    nc.vector.tensor_add(dst, dst, pv)
                                if m == 2 * r + 1 and dvc % 2 == 1:
                                    finalize_dh(r, dvc // 2)

    nc.finalize()
    return nc


def _make_in_maps(x, z, Wq, bq, Wk, bk, Wv, bv):
    bf = ml_dtypes.bfloat16
    x = np.asarray(x, dtype=np.float32)
    z = np.asarray(z, dtype=np.float32)
    wq_b = np.ascontiguousarray(np.asarray(Wq, dtype=np.float32).astype(bf))
    wk_b = np.ascontiguousarray(np.asarray(Wk, dtype=np.float32).astype(bf))
    wv_b = np.ascontiguousarray(np.asarray(Wv, dtype=np.float32).astype(bf))
    bq = np.asarray(bq, dtype=np.float32)
    bk = np.asarray(bk, dtype=np.float32)
    bv = np.asarray(bv, dtype=np.float32)
    iota = np.arange(QB, dtype=np.float32)
    in_maps = []
    for c in range(NCORES):
        blocks = [8 * p + c for p in range(NBLK)]
        x_blk = np.concatenate([x[b * QB : (b + 1) * QB] for b in blocks], axis=0)
        ig = np.stack([b * QB + iota for b in blocks], axis=0)
        in_maps.append(
            {
                "xT_blk": np.ascontiguousarray(x_blk.T.astype(bf)),
                "zT_blk": np.ascontiguousarray(
                    z[c * 1024 : (c + 1) * 1024].T.astype(bf)
                ),
                "wq": wq_b,
                "wk": wk_b,
                "wv": wv_b,
                "bq": bq,
                "bk": bk,
                "bv": bv,
                "ig_rows": np.ascontiguousarray(ig),
            }
        )
    return in_maps


def _unshard(results):
    full = np.empty((L, D), dtype=np.float32)
    for c in range(NCORES):
        o = results[c]["out"]
        for p in range(NBLK):
            b = 8 * p + c
            full[b * QB : (b + 1) * QB] = o[p * QB : (p + 1) * QB]
    return full


def kernel(x, z, Wq, bq, Wk, bk, Wv, bv):
    if "nc" not in _cache:
        t0 = time.time()
        _cache["nc"] = _build()
        _cache["build_s"] = time.time() - t0

    in_maps = _make_in_maps(x, z, Wq, bq, Wk, bk, Wv, bv)

    t0 = time.time()
    last_err = None
    for attempt in range(3):
        try:
            res = run_bass_kernel_spmd(
                _cache["nc"], in_maps, core_ids=list(range(NCORES))
            )
            break
        except Exception as e:  # transient NRT_EXEC_UNIT_UNRECOVERABLE after a
            last_err = e  # prior process exits; an immediate retry succeeds
            time.sleep(10)
    else:
        raise last_err
    _cache["run_s"] = time.time() - t0

    return _unshard(res.results)


def timed_run(in_maps, n_iter=3, pipelined=False):
    """Stage inputs on the 8 cores, run the kernel n_iter times, return
    (per-core results, list of wall seconds per on-device invocation)."""
    import jax
    from jax.experimental.shard_map import shard_map
    from jax.sharding import Mesh, NamedSharding, PartitionSpec

    from concourse import mybir as _mb
    from concourse.bass2jax import (
        _bass_exec_p,
        install_neuronx_cc_hook,
        partition_id_tensor,
    )

    nc = _cache["nc"]
    install_neuronx_cc_hook()

    partition_name = nc.partition_id_tensor.name if nc.partition_id_tensor else None
    in_names, out_names, out_avals, zero_outs = [], [], [], []
    for alloc in nc.m.functions[0].allocations:
        if not isinstance(alloc, _mb.MemoryLocationSet):
            continue
        name = alloc.memorylocations[0].name
        if alloc.kind == "ExternalInput":
            if name != partition_name:
                in_names.append(name)
        elif alloc.kind == "ExternalOutput":
            out_names.append(name)
            out_avals.append(
                jax.core.ShapedArray(tuple(alloc.tensor_shape), _mb.dt.np(alloc.dtype))
            )
            zero_outs.append(
                np.zeros(tuple(alloc.tensor_shape), _mb.dt.np(alloc.dtype))
            )
    n_params = len(in_names)
    n_outs = len(out_names)
    all_in_names = list(in_names) + out_names
    if partition_name is not None:
        all_in_names.append(partition_name)
    donate = tuple(range(n_params, n_params + n_outs))

    def _body(*args):
        operands = list(args)
        if partition_name is not None:
            operands.append(partition_id_tensor())
        outs = _bass_exec_p.bind(
            *operands,
            out_avals=tuple(out_avals),
            in_names=tuple(all_in_names),
            out_names=tuple(out_names),
            lowering_input_output_aliases=(),
            sim_require_finite=True,
            sim_require_nnan=True,
            nc=nc,
        )
        return tuple(outs)

    devices = jax.devices()[:NCORES]
    mesh = Mesh(np.asarray(devices), ("core",))
    spec = NamedSharding(mesh, PartitionSpec("core"))
    sharded = jax.jit(
        shard_map(
            _body,
            mesh=mesh,
            in_specs=(PartitionSpec("core"),) * (n_params + n_outs),
            out_specs=(PartitionSpec("core"),) * n_outs,
            check_rep=False,
        ),
        donate_argnums=donate,
        keep_unused=True,
    )

    concat_in = [
        jax.device_put(
            np.concatenate([np.asarray(in_maps[c][n]) for c in range(NCORES)], axis=0),
            spec,
        )
        for n in in_names
    ]
    zero_sets = [
        [
            jax.device_put(
                np.zeros((NCORES * zz.shape[0], *zz.shape[1:]), zz.dtype), spec
            )
            for zz in zero_outs
        ]
        for _ in range(n_iter)
    ]
    for a in concat_in:
        a.block_until_ready()
    for zs in zero_sets:
        for zz in zs:
            zz.block_until_ready()

    times = []
    out_arrs = None
    for it in range(n_iter):
        t0 = time.time()
        out_arrs = sharded(*concat_in, *zero_sets[it])
        for o in out_arrs:
            o.block_until_ready()
        times.append(time.time() - t0)

    def fresh_zero_sets(k):
        zs = [
            [
                jax.device_put(
                    np.zeros((NCORES * zz.shape[0], *zz.shape[1:]), zz.dtype), spec
                )
                for zz in zero_outs
            ]
            for _ in range(k)
        ]
        for zset in zs:
            for zz in zset:
                zz.block_until_ready()
        return zs

    for k in ((2, 8) if pipelined else ()):
        zsets = fresh_zero_sets(k)
        t0 = time.time()
        outs = [sharded(*concat_in, *zsets[i]) for i in range(k)]
        for oset in outs:
            for o in oset:
                o.block_until_ready()
        times.append((k, time.time() - t0))

    results = [
        {
            n: np.asarray(out_arrs[i]).reshape(NCORES, *out_avals[i].shape)[c]
            for i, n in enumerate(out_names)
        }
        for c in range(NCORES)
    ]
    return results, times


# revision 33
# speedup vs baseline: 1.0143x; 1.0143x over previous
"""Causal self-attention (L=8192, D=2048) on 8 TRN2 NeuronCores.

Sharding: core c owns eight 128-row query blocks with global ids {8p+c,
p=0..7} and KV rows [c*1024, (c+1)*1024).  For each 128-key tile the set of
positions still inside their causal range is a contiguous suffix of the
qt_sb columns, so one wide score/PV matmul (N up to 512) covers all of them
at once: causal over-compute drops to 1.125x of the ideal triangle with no
small-N matmul penalty.  Everything flows in bf16 (fp32 PSUM accumulation): the host passes
x^T/z^T pre-transposed bf16 so the device does no transposes, and K^T/V are
gathered in bf16, halving DMA and collective bytes.  The K^T and V AllGathers
are each split in two key-halves (via jb-outer / row-group-outer projection
loops) so four small collectives pipeline behind the remaining projection
compute instead of serializing at the phase boundary.  Phase 2
iterates chunk-outer/block-inner so each K/V chunk is loaded once per core;
scores are computed transposed (S^T = K^T-tile @ Q^T) so the softmax sum
reduces via a ones-matmul on the PE and P^T feeds the P@V matmul directly.
Each position finalizes (1/l scale + bv + store) right after its last causal
chunk, overlapping the epilogue with later chunks' compute.  No
max-subtraction: scores/sqrt(d) are O(+-6) for these inputs, well within exp
range.
"""

import math
import time
from contextlib import ExitStack

import ml_dtypes
import numpy as np

import concourse.bass as bass
import concourse.tile as tile
from concourse.masks import make_identity
from concourse import bacc, mybir
from concourse.bass_utils import run_bass_kernel_spmd

L = 8192
D = 2048  # d_x == d_attn == d_v
NCORES = 8
QB = 128  # query rows per block
NBLK = 8  # q-blocks per core
JC = 256  # kv keys per chunk
NCH = L // JC  # 32 chunks
NDT = D // 128  # 16 contraction tiles
SCALE = 1.0 / math.sqrt(D)

F32 = mybir.dt.float32
BF16 = mybir.dt.bfloat16

_cache = {}
_SIM_AG_TRAFFIC = True


def _build(repeat=1, sim=False):
    nc = bacc.Bacc("TRN2", num_devices=1 if sim else NCORES)

    xT = nc.dram_tensor("xT_blk", [D, NBLK * QB], BF16, kind="ExternalInput")
    zT = nc.dram_tensor("zT_blk", [D, 1024], BF16, kind="ExternalInput")
    wq = nc.dram_tensor("wq", [D, D], BF16, kind="ExternalInput")
    wk = nc.dram_tensor("wk", [D, D], BF16, kind="ExternalInput")
    wv = nc.dram_tensor("wv", [D, D], BF16, kind="ExternalInput")
    bq = nc.dram_tensor("bq", [D], F32, kind="ExternalInput")
    bk = nc.dram_tensor("bk", [D], F32, kind="ExternalInput")
    bv = nc.dram_tensor("bv", [D], F32, kind="ExternalInput")
    ig_rows = nc.dram_tensor("ig_rows", [NBLK, QB], F32, kind="ExternalInput")
    out = nc.dram_tensor("out", [NBLK * QB, D], F32, kind="ExternalOutput")

    shared = {} if sim else dict(addr_space="Shared")
    # K^T stored chunk-major: [chunk][d-sub 128][d-tile][key].  Split in two
    # key-halves per core so each half AllGathers as soon as it is projected.
    kt_locs = [
        nc.dram_tensor(f"kt_loc{h}", [2, 128, NDT, JC], BF16) for h in range(2)
    ]
    kt_gs = [
        nc.dram_tensor(f"kt_g{h}", [2 * NCORES, 128, NDT, JC], BF16, **shared)
        for h in range(2)
    ]
    # V split in key-row halves (local z rows 0:512 / 512:1024) so the first
    # half's gather overlaps the second half's projection, and attention's
    # early chunk pairs need only half A
    v_locs = [nc.dram_tensor(f"v_loc{h}", [512, D], BF16) for h in range(2)]
    v_gs = [
        nc.dram_tensor(f"v_g{h}", [L // 2, D], BF16, **shared) for h in range(2)
    ]

    def allgather(src, dst):
        if sim:
            if _SIM_AG_TRAFFIC:
                # emulate the gather's HBM traffic + dependency structure
                # (real collectives run on reserved SDMA rows)
                n = src.shape[0]
                for r in range(NCORES):
                    nc.gpsimd.dma_start(dst[r * n : (r + 1) * n], src[:])
            return
        nc.gpsimd.collective_compute(
            "AllGather",
            mybir.AluOpType.bypass,
            replica_groups=[list(range(NCORES))],
            ins=[src.ap().opt()],
            outs=[dst.ap().opt()],
        )

    with tile.TileContext(nc) as tc:
        with ExitStack() as consts:
            cp = consts.enter_context(tc.tile_pool(name="consts", bufs=1))
            ident2 = cp.tile([2, 2], F32)
            make_identity(nc, ident2)
            ones_f = cp.tile([128, 2], F32)
            nc.vector.memset(ones_f, 1.0)
            ones = cp.tile([128, 2], BF16)
            nc.vector.tensor_copy(ones, ones_f)
            # jg[p, k] = 128*k + p == global j index of partition p in j-tile k
            jg = cp.tile([128, L // 128], F32)
            nc.gpsimd.iota(
                jg,
                pattern=[[128, L // 128]],
                base=0,
                channel_multiplier=1,
                allow_small_or_imprecise_dtypes=True,
            )
            # biases first on the SWDGE queue: bk gates the first K-proj
            # store activation; the ig/bv broadcasts aren't needed until
            # phase 2
            bk_sb = cp.tile([128, NDT], F32, tag="bk")
            nc.gpsimd.dma_start(
                bk_sb, bass.AP(tensor=bk, offset=0, ap=[[1, 128], [128, NDT]])
            )
            bq_sb = cp.tile([128, NDT], F32, tag="bq")
            nc.gpsimd.dma_start(
                bq_sb, bass.AP(tensor=bq, offset=0, ap=[[1, 128], [128, NDT]])
            )
            # ig[p][:, f] = global i index of column f of block p (same all parts)
            igs = []
            for p in range(NBLK):
                igt = cp.tile([128, QB], F32, tag=f"ig{p}")
                nc.gpsimd.dma_start(
                    igt,
                    bass.AP(tensor=ig_rows, offset=p * QB, ap=[[0, 128], [1, QB]]),
                )
                igs.append(igt)
            bv_bc = cp.tile([128, D], F32)
            nc.gpsimd.dma_start(
                bv_bc, bass.AP(tensor=bv, offset=0, ap=[[0, 128], [1, D]])
            )
            qtp = consts.enter_context(tc.tile_pool(name="qt_sb", bufs=1))

            for _rep in range(repeat):
                qt_sb = qtp.tile([128, NDT, NBLK * QB], BF16, tag="qt")

                # ---------------- Phase 1: projections ----------------
                with ExitStack() as p1:
                    ztp = p1.enter_context(tc.tile_pool(name="zt", bufs=1))
                    xtp = p1.enter_context(tc.tile_pool(name="xt", bufs=1))
                    prp = p1.enter_context(
                        tc.tile_pool(name="prj_ps", bufs=3, space="PSUM")
                    )
                    stg = p1.enter_context(tc.tile_pool(name="stg", bufs=6))
                    wpp = p1.enter_context(tc.tile_pool(name="wpanel", bufs=3))

                    # first weight panel before the bulk loads: PE starts ~2us in
                    wp0 = wpp.tile([128, NDT, 256], BF16, tag="wp")
                    nc.sync.dma_start(
                        wp0, wk[:, 0:256].rearrange("(dt p) c -> p dt c", p=128)
                    )
                    # z^T / x^T resident, loaded in d-tile quarters so the first
                    # projection group starts as soon as its slices land
                    zt = ztp.tile([128, NDT, 1024], BF16)
                    for jh in range(2):
                        # half 0 feeds the first jb=0 groups (sync ring, right
                        # behind wp0); half 1 rides the idle SWDGE ring so the
                        # jb=0 weight panels stream uninterrupted
                        eng = nc.sync if jh == 0 else nc.gpsimd
                        eng.dma_start(
                            zt[:, :, 512 * jh : 512 * (jh + 1)],
                            zT[:, 512 * jh : 512 * (jh + 1)].rearrange(
                                "(t p) j -> p t j", p=128
                            ),
                        )
                    def proj_T(w_dram, b_sb, rhs3, jb, kt_dst=None, qt_dst=None,
                               first_wp=None):
                        # dst[t*128:(t+1)*128, :] = (W[:, t-cols]^T @ rhs_jb) + b[t]
                        for t2 in range(NDT // 2):
                            if first_wp is not None and t2 == 0:
                                wp = first_wp
                            else:
                                wp = wpp.tile([128, NDT, 256], BF16, tag="wp")
                                nc.sync.dma_start(
                                    wp,
                                    w_dram[:, t2 * 256 : (t2 + 1) * 256].rearrange(
                                        "(dt p) c -> p dt c", p=128
                                    ),
                                )
                            for th in range(2):
                                t = 2 * t2 + th
                                ps = prp.tile([128, 512], F32, tag="prj")
                                for dt in range(NDT):
                                    nc.tensor.matmul(
                                        ps,
                                        wp[:, dt, th * 128 : (th + 1) * 128],
                                        rhs3[:, dt, jb * 512 : (jb + 1) * 512],
                                        start=(dt == 0),
                                        stop=(dt == NDT - 1),
                                    )
                                if kt_dst is not None:
                                    st = stg.tile([128, 512], BF16, tag="stg")
                                    nc.scalar.activation(
                                        st,
                                        ps,
                                        mybir.ActivationFunctionType.Identity,
                                        bias=b_sb[:, t : t + 1],
                                    )
                                    for h in range(2):
                                        nc.scalar.dma_start(
                                            kt_dst[h][:, t, :],
                                            st[:, h * JC : (h + 1) * JC],
                                        )
                                else:
                                    nc.scalar.activation(
                                        qt_dst[:, t, jb * 512 : (jb + 1) * 512],
                                        ps,
                                        mybir.ActivationFunctionType.Identity,
                                        bias=b_sb[:, t : t + 1],
                                    )

                    # K first, in key-halves, so each half's gather overlaps
                    # the remaining projections
                    proj_T(wk, bk_sb, zt, jb=0, kt_dst=kt_locs[0], first_wp=wp0)
                    allgather(kt_locs[0], kt_gs[0])
                    proj_T(wk, bk_sb, zt, jb=1, kt_dst=kt_locs[1])
                    allgather(kt_locs[1], kt_gs[1])

                    # V = z @ Wv (natural layout); bv folded into the
                    # epilogue.  Row-group outer (panels reloaded per group)
                    # so each 512-key half gathers as soon as it is done.
                    for jg2 in range(2):
                        for cp4 in range(4):
                            wvh = wpp.tile([128, NDT, 512], BF16, tag="wvh")
                            nc.sync.dma_start(
                                wvh,
                                wv[:, cp4 * 512 : (cp4 + 1) * 512].rearrange(
                                    "(dt p) c -> p dt c", p=128
                                ),
                            )
                            for jt in range(4):
                                jtg = jg2 * 4 + jt
                                ps = prp.tile([128, 512], F32, tag="prj")
                                for dt in range(NDT):
                                    nc.tensor.matmul(
                                        ps,
                                        zt[:, dt, jtg * 128 : (jtg + 1) * 128],
                                        wvh[:, dt, :],
                                        start=(dt == 0),
                                        stop=(dt == NDT - 1),
                                    )
                                st = stg.tile([128, 512], BF16, tag="stg")
                                nc.vector.tensor_copy(st, ps)
                                nc.scalar.dma_start(
                                    v_locs[jg2][
                                        jt * 128 : (jt + 1) * 128,
                                        cp4 * 512 : (cp4 + 1) * 512,
                                    ],
                                    st,
                                )
                        allgather(v_locs[jg2], v_gs[jg2])

                    # Q last: its output stays in SBUF for phase 2.  x^T is
                    # loaded here (not earlier) so its 4MB doesn't clog the
                    # DMA queue ahead of the K/V weight panels.
                    xt = xtp.tile([128, NDT, NBLK * QB], BF16)
                    for q in range(4):
                        nc.sync.dma_start(
                            xt[:, 4 * q : 4 * (q + 1), :],
                            xT[512 * q : 512 * (q + 1), :].rearrange(
                                "(t p) j -> p t j", p=128
                            ),
                        )
                    for jb in range(2):
                        proj_T(wq, bq_sb, xt, jb=jb, qt_dst=qt_sb)

                # ---------------- Phase 2: causal attention ----------------
                with ExitStack() as p2:
                    ktp = p2.enter_context(tc.tile_pool(name="kt", bufs=4))
                    vcp = p2.enter_context(tc.tile_pool(name="vc", bufs=4))
                    ptp = p2.enter_context(tc.tile_pool(name="pt", bufs=10))
                    mkp = p2.enter_context(tc.tile_pool(name="mk", bufs=2))
                    acp = p2.enter_context(tc.tile_pool(name="acc", bufs=1))
                    fin = p2.enter_context(tc.tile_pool(name="fin", bufs=2))
                    stp = p2.enter_context(
                        tc.tile_pool(name="st_ps", bufs=3, space="PSUM")
                    )
                    pvp = p2.enter_context(
                        tc.tile_pool(name="pv_ps", bufs=3, space="PSUM")
                    )
                    llp = p2.enter_context(
                        tc.tile_pool(name="l_ps", bufs=1, space="PSUM")
                    )

                    accs = [
                        acp.tile([128, D], F32, tag=f"acc{r}", name=f"acc{r}")
                        for r in range(NBLK)
                    ]
                    l2_acc = acp.tile([2, NBLK * QB], F32, tag="lacc")

                    recips = {}

                    def finalize_dh(p, dh):
                        # ACT/DVE pipeline in 512-col pieces; a contiguous
                        # half-row store on the sync ring (idle at tail),
                        # emitted right after the PV spill completing it
                        if p not in recips:
                            lt_ps = llp.tile([128, 2], F32, tag="lt")
                            nc.tensor.transpose(
                                lt_ps, l2_acc[:, p * QB : (p + 1) * QB], ident2
                            )
                            recip = fin.tile([128, 1], F32, tag="recip")
                            nc.vector.reciprocal(recip, lt_ps[:, 0:1])
                            recips[p] = recip
                        recip = recips[p]
                        of = fin.tile([128, D // 2], F32, tag="of")
                        for dq in range(2):
                            sl = slice(
                                dh * 1024 + dq * 512,
                                dh * 1024 + (dq + 1) * 512,
                            )
                            osl = slice(dq * 512, (dq + 1) * 512)
                            nc.scalar.activation(
                                of[:, osl],
                                accs[p][:, sl],
                                mybir.ActivationFunctionType.Copy,
                                scale=recip[:, 0:1],
                            )
                            nc.vector.tensor_add(
                                of[:, osl], of[:, osl], bv_bc[:, sl]
                            )
                        nc.sync.dma_start(
                            out[
                                p * QB : (p + 1) * QB,
                                dh * 1024 : (dh + 1) * 1024,
                            ],
                            of,
                        )

                    # even groups read the A-halves of kt_g/v_g, odd
                    # groups the B-halves; processing each quad's even
                    # groups first doubles the slack for the B-half gathers
                    # to land (group semantics are order-independent: group
                    # 0 still comes first and initializes every accumulator,
                    # and position r's last touching group stays 2r+1)
                    order = [
                        q * 4 + g for q in range(NCH // 8) for g in (0, 2, 1, 3)
                    ]
                    for m in order:
                        # group m: stored chunks (2m, 2m+1) = j-tiles
                        # [4m, 4m+4), all with the same active-position set
                        # [pmin, 8) since the range never crosses an
                        # 8-j-tile boundary
                        pmin = m // 2
                        W = (NBLK - pmin) * QB
                        nW = (W + 511) // 512
                        js = (2 * m, 2 * m + 1)
                        kts = []
                        vcs = []
                        for ci, j in enumerate(js):
                            kt = ktp.tile([128, NDT, JC], BF16, tag="kt")
                            nc.sync.dma_start(
                                kt, kt_gs[(j % 4) // 2][2 * (j // 4) + (j % 2)]
                            )
                            kts.append(kt)
                            mj = j % 4
                            ro = 512 * (j // 4) + 256 * (mj % 2)
                            vch = vcp.tile([128, 2, D], BF16, tag="vc", name="vc")
                            nc.sync.dma_start(
                                vch,
                                v_gs[mj // 2][ro : ro + 256, :].rearrange(
                                    "(jt p) d -> p jt d", p=128
                                ),
                            )
                            vcs.append(vch)
                        # scores + exp + mask for all 4 j-tiles, fused over
                        # the active positions (one wide MM per 512 piece)
                        pts = []  # pts[2*ci+jt][nh]
                        for ci in range(2):
                            for jt in range(2):
                                pieces = []
                                for nh in range(nW):
                                    w = min(512, W - 512 * nh)
                                    st_ps = stp.tile([128, 512], F32, tag="st")
                                    q0 = pmin * QB + 512 * nh
                                    for dt in range(NDT):
                                        nc.tensor.matmul(
                                            st_ps[:, :w],
                                            kts[ci][
                                                :, dt, jt * 128 : (jt + 1) * 128
                                            ],
                                            qt_sb[:, dt, q0 : q0 + w],
                                            start=(dt == 0),
                                            stop=(dt == NDT - 1),
                                        )
                                    pt = ptp.tile([128, 512], BF16, tag="pt")
                                    nc.scalar.activation(
                                        pt[:, :w],
                                        st_ps[:, :w],
                                        mybir.ActivationFunctionType.Exp,
                                        scale=SCALE,
                                    )
                                    if nh == 0:
                                        # position pmin is inside its causal
                                        # band for every j-tile of this group
                                        k = 4 * m + 2 * ci + jt
                                        mk = mkp.tile([128, QB], BF16, tag="mk")
                                        nc.vector.tensor_scalar(
                                            mk,
                                            igs[pmin],
                                            jg[:, k : k + 1],
                                            None,
                                            mybir.AluOpType.is_ge,
                                        )
                                        nc.vector.tensor_mul(
                                            pt[:, :QB], pt[:, :QB], mk
                                        )
                                    pieces.append(pt)
                                pts.append(pieces)

                        def ptsl(idx, r):
                            off = (r - pmin) * QB
                            return pts[idx][off // 512][
                                :, off % 512 : off % 512 + QB
                            ]

                        for nh in range(nW):
                            w = min(512, W - 512 * nh)
                            l_ps = llp.tile([2, 512], F32, tag="l")
                            for idx in range(4):
                                nc.tensor.matmul(
                                    l_ps[:, :w],
                                    ones,
                                    pts[idx][nh][:, :w],
                                    start=(idx == 0),
                                    stop=(idx == 3),
                                )
                            q0 = pmin * QB + 512 * nh
                            dst = l2_acc[:, q0 : q0 + w]
                            if m == 0:
                                nc.vector.tensor_copy(dst, l_ps[:, :w])
                            else:
                                nc.vector.tensor_add(dst, dst, l_ps[:, :w])
                        for r in range(pmin, NBLK):
                            for dvc in range(4):
                                pv = pvp.tile([128, 512], F32, tag="pv")
                                for ci in range(2):
                                    for jt in range(2):
                                        nc.tensor.matmul(
                                            pv,
                                            ptsl(2 * ci + jt, r),
                                            vcs[ci][
                                                :,
                                                jt,
                                                dvc * 512 : (dvc + 1) * 512,
                                            ],
                                            start=(ci == 0 and jt == 0),
                                            stop=(ci == 1 and jt == 1),
                                        )
                                dst = accs[r][:, dvc * 512 : (dvc + 1) * 512]
                                if m == 0:
                                    nc.vector.tensor_copy(dst, pv)
                                else:
                                    nc.vector.tensor_add(dst, dst, pv)
                                if m == 2 * r + 1 and dvc % 2 == 1:
                                    finalize_dh(r, dvc // 2)

    nc.finalize()
    return nc


def _make_in_maps(x, z, Wq, bq, Wk, bk, Wv, bv):
    bf = ml_dtypes.bfloat16
    x = np.asarray(x, dtype=np.float32)
    z = np.asarray(z, dtype=np.float32)
    wq_b = np.ascontiguousarray(np.asarray(Wq, dtype=np.float32).astype(bf))
    wk_b = np.ascontiguousarray(np.asarray(Wk, dtype=np.float32).astype(bf))
    wv_b = np.ascontiguousarray(np.asarray(Wv, dtype=np.float32).astype(bf))
    bq = np.asarray(bq, dtype=np.float32)
    bk = np.asarray(bk, dtype=np.float32)
    bv = np.asarray(bv, dtype=np.float32)
    iota = np.arange(QB, dtype=np.float32)
    in_maps = []
    for c in range(NCORES):
        blocks = [8 * p + c for p in range(NBLK)]
        x_blk = np.concatenate([x[b * QB : (b + 1) * QB] for b in blocks], axis=0)
        ig = np.stack([b * QB + iota for b in blocks], axis=0)
        in_maps.append(
            {
                "xT_blk": np.ascontiguousarray(x_blk.T.astype(bf)),
                "zT_blk": np.ascontiguousarray(
                    z[c * 1024 : (c + 1) * 1024].T.astype(bf)
                ),
                "wq": wq_b,
                "wk": wk_b,
                "wv": wv_b,
                "bq": bq,
                "bk": bk,
                "bv": bv,
                "ig_rows": np.ascontiguousarray(ig),
            }
        )
    return in_maps


def _unshard(results):
    full = np.empty((L, D), dtype=np.float32)
    for c in range(NCORES):
        o = results[c]["out"]
        for p in range(NBLK):
            b = 8 * p + c
            full[b * QB : (b + 1) * QB] = o[p * QB : (p + 1) * QB]
    return full


def kernel(x, z, Wq, bq, Wk, bk, Wv, bv):
    if "nc" not in _cache:
        t0 = time.time()
        _cache["nc"] = _build()
        _cache["build_s"] = time.time() - t0

    in_maps = _make_in_maps(x, z, Wq, bq, Wk, bk, Wv, bv)

    t0 = time.time()
    last_err = None
    for attempt in range(3):
        try:
            res = run_bass_kernel_spmd(
                _cache["nc"], in_maps, core_ids=list(range(NCORES))
            )
            break
        except Exception as e:  # transient NRT_EXEC_UNIT_UNRECOVERABLE after a
            last_err = e  # prior process exits; an immediate retry succeeds
            time.sleep(10)
    else:
        raise last_err
    _cache["run_s"] = time.time() - t0

    return _unshard(res.results)


def timed_run(in_maps, n_iter=3, pipelined=False):
    """Stage inputs on the 8 cores, run the kernel n_iter times, return
    (per-core results, list of wall seconds per on-device invocation)."""
    import jax
    from jax.experimental.shard_map import shard_map
    from jax.sharding import Mesh, NamedSharding, PartitionSpec

    from concourse import mybir as _mb
    from concourse.bass2jax import (
        _bass_exec_p,
        install_neuronx_cc_hook,
        partition_id_tensor,
    )

    nc = _cache["nc"]
    install_neuronx_cc_hook()

    partition_name = nc.partition_id_tensor.name if nc.partition_id_tensor else None
    in_names, out_names, out_avals, zero_outs = [], [], [], []
    for alloc in nc.m.functions[0].allocations:
        if not isinstance(alloc, _mb.MemoryLocationSet):
            continue
        name = alloc.memorylocations[0].name
        if alloc.kind == "ExternalInput":
            if name != partition_name:
                in_names.append(name)
        elif alloc.kind == "ExternalOutput":
            out_names.append(name)
            out_avals.append(
                jax.core.ShapedArray(tuple(alloc.tensor_shape), _mb.dt.np(alloc.dtype))
            )
            zero_outs.append(
                np.zeros(tuple(alloc.tensor_shape), _mb.dt.np(alloc.dtype))
            )
    n_params = len(in_names)
    n_outs = len(out_names)
    all_in_names = list(in_names) + out_names
    if partition_name is not None:
        all_in_names.append(partition_name)
    donate = tuple(range(n_params, n_params + n_outs))

    def _body(*args):
        operands = list(args)
        if partition_name is not None:
            operands.append(partition_id_tensor())
        outs = _bass_exec_p.bind(
            *operands,
            out_avals=tuple(out_avals),
            in_names=tuple(all_in_names),
            out_names=tuple(out_names),
            lowering_input_output_aliases=(),
            sim_require_finite=True,
            sim_require_nnan=True,
            nc=nc,
        )
        return tuple(outs)

    devices = jax.devices()[:NCORES]
    mesh = Mesh(np.asarray(devices), ("core",))
    spec = NamedSharding(mesh, PartitionSpec("core"))
    sharded = jax.jit(
        shard_map(
            _body,
            mesh=mesh,
            in_specs=(PartitionSpec("core"),) * (n_params + n_outs),
            out_specs=(PartitionSpec("core"),) * n_outs,
            check_rep=False,
        ),
        donate_argnums=donate,
        keep_unused=True,
    )

    concat_in = [
        jax.device_put(
            np.concatenate([np.asarray(in_maps[c][n]) for c in range(NCORES)], axis=0),
            spec,
        )
        for n in in_names
    ]
    zero_sets = [
        [
            jax.device_put(
                np.zeros((NCORES * zz.shape[0], *zz.shape[1:]), zz.dtype), spec
            )
            for zz in zero_outs
        ]
        for _ in range(n_iter)
    ]
    for a in concat_in:
        a.block_until_ready()
    for zs in zero_sets:
        for zz in zs:
            zz.block_until_ready()

    times = []
    out_arrs = None
    for it in range(n_iter):
        t0 = time.time()
        out_arrs = sharded(*concat_in, *zero_sets[it])
        for o in out_arrs:
            o.block_until_ready()
        times.append(time.time() - t0)

    def fresh_zero_sets(k):
        zs = [
            [
                jax.device_put(
                    np.zeros((NCORES * zz.shape[0], *zz.shape[1:]), zz.dtype), spec
                )
                for zz in zero_outs
            ]
            for _ in range(k)
        ]
        for zset in zs:
            for zz in zset:
                zz.block_until_ready()
        return zs

    for k in ((2, 8) if pipelined else ()):
        zsets = fresh_zero_sets(k)
        t0 = time.time()
        outs = [sharded(*concat_in, *zsets[i]) for i in range(k)]
        for oset in outs:
            for o in oset:
                o.block_until_ready()
        times.append((k, time.time() - t0))

    results = [
        {
            n: np.asarray(out_arrs[i]).reshape(NCORES, *out_avals[i].shape)[c]
            for i, n in enumerate(out_names)
        }
        for c in range(NCORES)
    ]
    return results, times
